# revision 34
# baseline (speedup 1.0000x reference)
"""NNUE evaluation kernel for Trainium2 (8 NeuronCores, data-parallel batch).

reference math:
    wh = clip(white @ W_ft.T, 0, 1)        # [B, 256]
    bh = clip(black @ W_ft.T, 0, 1)        # [B, 256]
    x  = concat(wh, bh)                    # [B, 512]
    x  = relu(x @ W1.T + b1); x = relu(x @ W2.T + b2)
    ev = (x @ W3.T + b3) * stm[:, None]    # [B, 1]

Strategy: shard B=4096 across 8 cores (512 rows each), data-parallel, no
collectives. All GEMM operands are cast to fp16 on the host (rel err
1.7e-3 vs the 2e-2 gate; fp8 e4m3 fails at ~1e-1 because the 40960-term
contraction amplifies quantization noise by sqrt(K)) and the features
are host-transposed/swizzled into the [k, b] layout the PE wants, so the
kernel is pure matmul: no on-chip transposes. Per core the two
[512, 40960] feature GEMMs read 105 MB of fp16 from HBM (~275-300 us at
the 340-400 GB/s per-NC rate) and cost 1280 N=512 matmuls x 216 ns on
the PE (~277 us at the 78.6 TF/s bf16-class rate) -- balanced at the
joint roofline. Features/weights stream in k-slabs of 4 k-tiles with
bufs=8 (DMA never waits on buffer-free), split across both HWDGE rings
(white on sync, black on scalar, W_ft alternating). W_ft.T tiles are
stationary, feat.T [k, b=512] moving, accumulating out.T [h, b] in PSUM
across all 320 k-tiles (4 banks: 2 sides x 2 h-tiles). Dummy warm-up
matmuls cover the ~8 us HWDGE bring-up so the HAM clock gate is at
2.4 GHz when real matmuls start. The clip is fused into the PSUM
evacuation (white's evacuation hides under black's final matmuls), the
tiny MLP stays in transposed [features, batch] layout, and b3 is folded
into the last matmul via a ones-row.

This walrus build rejects instructions with >1 sync wait, so a post-pass
(_split_multi_waits) redistributes Tile-emitted waits onto single-wait
no-ops.
"""

import sys
import types

import numpy as np


def _inject_ntff_hook():
    """Register the axon NTFF profile hook if this image's antenv lacks it."""
    try:
        import antenv.axon_hooks  # noqa: F401
        return
    except ImportError:
        pass
    try:
        import trn_agent_boot.trn_boot as tb
        hook = tb._ntff_profile_via_ctypes("/opt/axon/libaxon_pjrt.so")
    except Exception:
        hook = None
    mod = types.ModuleType("antenv.axon_hooks")
    mod.get_axon_ntff_profile_hook = lambda: hook
    mod.set_axon_ntff_profile_hook = lambda h: None
    sys.modules["antenv.axon_hooks"] = mod


_inject_ntff_hook()

import concourse.bass as bass
import concourse.mybir as mybir
from concourse.tile import TileContext

N_CORES = 8
B = 4096
BS = B // N_CORES          # 512 batch rows per core
IN = 40960                 # feature count (contraction dim)
H = 256                    # hidden per perspective
NKT = 4                    # k-tiles per slab
KC = NKT * 128             # k-slab width: 512
NSLAB = IN // KC           # 80
NKTOT = IN // 128          # 320 k-tiles total

F32 = mybir.dt.float32
F16 = mybir.dt.float16


def _split_multi_waits(nc: bass.Bass) -> None:
    """This walrus build rejects instructions carrying more than one sync
    wait. Split any such instruction: emit single-wait no-ops on the same
    engine immediately before it (same engine stream => same semantics)."""
    for f in nc.m.functions:
        for bb in f.blocks:
            new_insts = []
            changed = False
            for inst in bb.instructions:
                si = inst.sync_info
                waits = list(si.on_wait) if si is not None and si.on_wait else []
                if len(waits) > 1:
                    changed = True
                    for i, w in enumerate(waits[:-1]):
                        nop = mybir.InstNoOp(
                            name=f"{inst.name}-sw{i}", ins=[], outs=[]
                        )
                        nop.engine = inst.engine
                        nop.sync_info = mybir.SyncInfo(on_wait=[w], on_update=[])
                        nc.register_instruction(nop)
                        new_insts.append(nop)
                    inst.sync_info = mybir.SyncInfo(
                        on_wait=[waits[-1]],
                        on_update=list(si.on_update) if si.on_update else [],
                    )
                new_insts.append(inst)
            if changed:
                bb.instructions = new_insts


def build_kernel(mm_f32r: bool = True, tr_f32r: bool = True) -> bass.Bass:
    nc = bass.Bass()

    # Features arrive host-transposed+swizzled fp16: [128, NKTOT*BS] where
    # row p, columns [kt*BS : (kt+1)*BS] hold feat.T[kt*128 + p, :]. Each
    # k-slab DMA reads NKT*BS*2 = 4 KB contiguous per partition.
    wf = nc.dram_tensor("white_fT", [128, NKTOT * BS], F16, kind="ExternalInput")
    bf = nc.dram_tensor("black_fT", [128, NKTOT * BS], F16, kind="ExternalInput")
    # W_ft.T swizzled the same way: [128, NKTOT*H], 4 KB/partition per slab.
    w_ftTs = nc.dram_tensor("W_ftTs", [128, NKTOT * H], F16, kind="ExternalInput")
    w1Ts = nc.dram_tensor("W1Ts", [128, 128], F16, kind="ExternalInput")
    b1 = nc.dram_tensor("b1", [32, 1], F32, kind="ExternalInput")
    w2T = nc.dram_tensor("W2T", [32, 32], F16, kind="ExternalInput")
    b2 = nc.dram_tensor("b2", [32, 1], F32, kind="ExternalInput")
    # W3T has b3 folded in as a 33rd row (paired with a ones-row in h2)
    w3T = nc.dram_tensor("W3T", [33, 1], F16, kind="ExternalInput")
    stm = nc.dram_tensor("side_to_move", [1, BS], F32, kind="ExternalInput")
    out = nc.dram_tensor("evaluation", [1, BS], F32, kind="ExternalOutput")

    feats = [wf, bf]

    with TileContext(nc) as tc:
        with (
            tc.tile_pool(name="ot_psum", bufs=1, space="PSUM") as ot_pool,
            tc.tile_pool(name="mlp", bufs=1) as mlp,
        ):
            # out.T accumulators: [h-tile 128, b 512] x (2 sides x 2 h-tiles)
            ot = [
                ot_pool.tile([128, BS], F32, tag=f"ot{i}", name=f"ot{i}")
                for i in range(4)
            ]
            xt = []  # clipped fp16 copies, filled during the last slab

            # ---- PE warm-up: the HAM clock gate defaults to 1.2 GHz and
            # only lifts to 2.4 GHz after ~3.4us of sustained PE activity.
            # Burn that window on dummy matmuls while the first feature
            # slabs are still in flight, so real matmuls start warm.
            with (
                tc.tile_pool(name="warm", bufs=1) as warm_pool,
                tc.tile_pool(name="warm_psum", bufs=1, space="PSUM") as wp_pool,
            ):
                dum_w = warm_pool.tile([128, 128], F16)
                nc.vector.memset(dum_w[:], 0.0)
                dum_f = warm_pool.tile([128, BS], F16)
                nc.vector.memset(dum_f[:], 0.0)
                # ~12 cold (427ns) + 2 warm (216ns) spans ~5.6us; the first
                # feature slab lands ~7.5us, leaving a <2us idle gap --
                # short enough that the clock gate stays at 8/8.
                dum_o = wp_pool.tile([128, BS], F32)
                for _ in range(14):
                    nc.tensor.matmul(
                        dum_o, dum_w[:], dum_f[:], start=True, stop=True
                    )

            # ---- main loop: feature-transformer GEMMs ----
            # slab widths in k-tiles: two small warmup slabs so the PE
            # starts early, then uniform NKT-wide slabs.
            widths = [1, 1, 2] + [NKT] * ((NKTOT - 4) // NKT)
            assert sum(widths) == NKTOT
            with (
                tc.tile_pool(name="fslab", bufs=8) as fslab_pool,
                tc.tile_pool(name="wt", bufs=8) as wt_pool,
                tc.tile_pool(name="pre", bufs=1) as pre_pool,
            ):
                # white features ride the sync HWDGE ring, black the
                # scalar ring; W_ft alternates so both rings carry ~52 MB.
                f_eng = [nc.sync, nc.scalar]
                kt0 = 0
                for s, w in enumerate(widths):
                    pre = w != NKT
                    pool = pre_pool if pre else fslab_pool
                    wpool = pre_pool if pre else wt_pool
                    wt = wpool.tile([128, w, H], F16, tag=f"wt{s}" if pre else "wt",
                                    name="wt")
                    f_eng[s % 2].dma_start(
                        out=wt[:],
                        in_=w_ftTs[:, kt0 * H:(kt0 + w) * H],
                    )
                    fsl = []
                    for side in range(2):
                        f_t = pool.tile(
                            [128, w, BS], F16,
                            tag=f"pre{side}_{s}" if pre else f"fslab{side}",
                            name=f"fsl{side}",
                        )
                        f_eng[side].dma_start(
                            out=f_t[:],
                            in_=feats[side][
                                :, kt0 * BS:(kt0 + w) * BS
                            ],
                        )
                        fsl.append(f_t)

                    last_slab = s == len(widths) - 1
                    if not last_slab:
                        for kt in range(w):
                            first = kt0 == 0 and kt == 0
                            for h in range(2):
                                for side in range(2):
                                    nc.tensor.matmul(
                                        ot[side * 2 + h],
                                        wt[:, kt, h * 128:(h + 1) * 128],
                                        fsl[side][:, kt, :],
                                        start=first,
                                        stop=False,
                                    )
                    else:
                        # final slab: finish white first, evacuate its
                        # PSUM banks while black's last matmuls run.
                        for side in range(2):
                            for kt in range(w):
                                for h in range(2):
                                    nc.tensor.matmul(
                                        ot[side * 2 + h],
                                        wt[:, kt, h * 128:(h + 1) * 128],
                                        fsl[side][:, kt, :],
                                        start=False,
                                        stop=kt == w - 1,
                                    )
                            for i in range(2 * side, 2 * side + 2):
                                t = mlp.tile([128, BS], F16, tag=f"xt{i}",
                                             name="xt")
                                # white: one full evac (hides under
                                # black's matmuls). black: halves, so
                                # the MLP can start on half 0 while
                                # half 1 still evacuates.
                                nsp = 1 if side == 0 else 2
                                for sp in range(nsp):
                                    sl = slice(sp * (BS // nsp),
                                               (sp + 1) * (BS // nsp))
                                    nc.vector.tensor_scalar(
                                        out=t[:, sl], in0=ot[i][:, sl],
                                        scalar1=0.0, scalar2=1.0,
                                        op0=mybir.AluOpType.max,
                                        op1=mybir.AluOpType.min,
                                    )
                                xt.append(t)
                    kt0 += w

            # ---- MLP weight prep (emitted late so these DMAs schedule
            # behind the feature stream, not ahead of it) ----
            w1t = mlp.tile([128, 4, 32], F16)
            nc.scalar.dma_start(out=w1t[:], in_=w1Ts[:, :])
            w2t = mlp.tile([32, 32], F16)
            nc.scalar.dma_start(out=w2t[:], in_=w2T[:, :])
            w3t = mlp.tile([33, 1], F16)
            nc.scalar.dma_start(out=w3t[:], in_=w3T[:, :])
            b1_sb = mlp.tile([32, 1], F32)
            nc.scalar.dma_start(out=b1_sb[:], in_=b1[:, :])
            b2_sb = mlp.tile([32, 1], F32)
            nc.scalar.dma_start(out=b2_sb[:], in_=b2[:, :])
            stm_sb = mlp.tile([1, BS], F32)
            nc.scalar.dma_start(out=stm_sb[:], in_=stm[:, :])
            # h2 carries a ones-row (partition 32) so the final matmul
            # against [W3.T; b3] folds the bias in.
            h2 = mlp.tile([33, BS], F16)
            nc.vector.memset(h2[32:33, :], 1.0)

            # ---- MLP (transposed layout throughout; xt built above).
            # The whole chain runs in two 256-column halves with
            # independent PSUM groups, so DVE evacuations of one half
            # pipeline with PE matmuls of the other. ----
            with tc.tile_pool(name="mlp2_psum", bufs=1, space="PSUM") as mpp2:
                h1p = mpp2.tile([32, BS], F32, tag="h1")
                h1 = mlp.tile([32, BS], F16)
                h2p = mpp2.tile([32, BS], F32, tag="h2")
                evp = mpp2.tile([1, BS], F32, tag="ev")
                HB = BS // 2
                for hf in range(2):
                    sl = slice(hf * HB, (hf + 1) * HB)
                    for kt in range(4):
                        nc.tensor.matmul(
                            h1p[:, sl], w1t[:, kt, :], xt[kt][:, sl],
                            start=kt == 0, stop=kt == 3,
                        )
                    nc.vector.tensor_scalar(
                        out=h1[:, sl], in0=h1p[:, sl], scalar1=b1_sb[:, :],
                        scalar2=0.0,
                        op0=mybir.AluOpType.add, op1=mybir.AluOpType.max,
                    )
                    nc.tensor.matmul(
                        h2p[:, sl], w2t[:], h1[:, sl], start=True, stop=True
                    )
                    nc.vector.tensor_scalar(
                        out=h2[0:32, sl], in0=h2p[:, sl], scalar1=b2_sb[:, :],
                        scalar2=0.0,
                        op0=mybir.AluOpType.add, op1=mybir.AluOpType.max,
                    )
                    nc.tensor.matmul(
                        evp[:, sl], w3t[:], h2[:, sl], start=True, stop=True
                    )
                evs = mlp.tile([1, BS], F32)
                nc.vector.tensor_mul(out=evs[:], in0=evp[:], in1=stm_sb[:])
                nc.sync.dma_start(out=out[:, :], in_=evs[:])

    _split_multi_waits(nc)
    return nc


_NC_CACHE: dict = {}


def _get_nc(mm_f32r: bool = True, tr_f32r: bool = True) -> bass.Bass:
    key = (mm_f32r, tr_f32r)
    if key not in _NC_CACHE:
        _NC_CACHE[key] = build_kernel(mm_f32r=mm_f32r, tr_f32r=tr_f32r)
    return _NC_CACHE[key]


def _swizzle_T(arr_f16: np.ndarray, ncols: int) -> np.ndarray:
    """[rows, IN] fp16 -> [128, NKTOT*rows] where row p, cols
    [kt*rows:(kt+1)*rows] = arr.T[kt*128 + p, :]."""
    rows = arr_f16.shape[0]
    assert arr_f16.shape == (rows, IN) and ncols == rows
    return np.ascontiguousarray(
        arr_f16.reshape(rows, NKTOT, 128).transpose(2, 1, 0)
    ).reshape(128, NKTOT * rows)


def make_in_maps(inputs: dict) -> list:
    """Shard full inputs into per-core input maps (fp16, transposed)."""
    wf = np.asarray(inputs["white_features"]).astype(np.float16)
    bf = np.asarray(inputs["black_features"]).astype(np.float16)
    stm = np.ascontiguousarray(inputs["side_to_move"], dtype=np.float32)
    w_ftTs = _swizzle_T(
        np.asarray(inputs["W_ft"], dtype=np.float32).astype(np.float16), H)
    w1T = np.asarray(inputs["W1"], dtype=np.float32).astype(np.float16).T
    w1Ts = np.ascontiguousarray(
        w1T.reshape(4, 128, 32).transpose(1, 0, 2)).reshape(128, 128)
    w2T = np.ascontiguousarray(
        np.asarray(inputs["W2"], dtype=np.float32).astype(np.float16).T)
    w3T = np.concatenate([
        np.asarray(inputs["W3"], dtype=np.float32).astype(np.float16).T,
        np.asarray(inputs["b3"], dtype=np.float32).astype(np.float16)
        .reshape(1, 1),
    ], axis=0)  # [33, 1]: W3.T with b3 folded in
    maps = []
    for c in range(N_CORES):
        sl = slice(c * BS, (c + 1) * BS)
        maps.append({
            "white_fT": _swizzle_T(wf[sl], BS),
            "black_fT": _swizzle_T(bf[sl], BS),
            "side_to_move": stm[sl].reshape(1, BS),
            "W_ftTs": w_ftTs,
            "W1Ts": w1Ts,
            "b1": np.ascontiguousarray(inputs["b1"], dtype=np.float32).reshape(32, 1),
            "W2T": w2T,
            "b2": np.ascontiguousarray(inputs["b2"], dtype=np.float32).reshape(32, 1),
            "W3T": w3T,
        })
    return maps


def run(inputs: dict, trace: bool = False, mm_f32r: bool = True,
        tr_f32r: bool = True):
    """Run on all 8 cores; returns (full_output [4096,1] fp32, BassKernelResults)."""
    from concourse.bass_utils import run_bass_kernel_spmd

    nc = _get_nc(mm_f32r=mm_f32r, tr_f32r=tr_f32r)
    res = run_bass_kernel_spmd(
        nc, make_in_maps(inputs), core_ids=list(range(N_CORES)), trace=trace
    )
    full = np.concatenate(
        [res.results[c]["evaluation"].reshape(BS, 1) for c in range(N_CORES)],
        axis=0,
    ).astype(np.float32)
    return full, res


def kernel(**inputs) -> np.ndarray:
    return run(inputs, trace=False)[0]


if __name__ == "__main__":
    rng = np.random.default_rng(0)
    ins = {
        "white_features": rng.random((B, IN), dtype=np.float32),
        "black_features": rng.random((B, IN), dtype=np.float32),
        "side_to_move": np.ones((B,), dtype=np.float32),
        "W_ft": (0.1 * rng.standard_normal((H, IN))).astype(np.float32),
        "W1": (0.06 * rng.standard_normal((32, 2 * H))).astype(np.float32),
        "b1": np.zeros(32, np.float32),
        "W2": (0.17 * rng.standard_normal((32, 32))).astype(np.float32),
        "b2": np.zeros(32, np.float32),
        "W3": (0.24 * rng.standard_normal((1, 32))).astype(np.float32),
        "b3": np.zeros(1, np.float32),
    }
    out = kernel(**ins)
    # host reference
    whr = np.clip(ins["white_features"] @ ins["W_ft"].T, 0, 1)
    bhr = np.clip(ins["black_features"] @ ins["W_ft"].T, 0, 1)
    x = np.concatenate([whr, bhr], axis=1)
    x = np.maximum(x @ ins["W1"].T + ins["b1"], 0)
    x = np.maximum(x @ ins["W2"].T + ins["b2"], 0)
    ref = (x @ ins["W3"].T + ins["b3"]) * ins["side_to_move"][:, None]
    rel = np.linalg.norm(out - ref) / np.linalg.norm(ref)
    print("rel err:", rel)


# revision 36
# speedup vs baseline: 1.0675x; 1.0675x over previous
"""NNUE evaluation kernel for Trainium2 (8 NeuronCores, data-parallel batch).

reference math:
    wh = clip(white @ W_ft.T, 0, 1)        # [B, 256]
    bh = clip(black @ W_ft.T, 0, 1)        # [B, 256]
    x  = concat(wh, bh)                    # [B, 512]
    x  = relu(x @ W1.T + b1); x = relu(x @ W2.T + b2)
    ev = (x @ W3.T + b3) * stm[:, None]    # [B, 1]

Strategy: shard B=4096 across 8 cores (512 rows each), data-parallel, no
collectives. All GEMM operands are cast to fp16 on the host (rel err
1.7e-3 vs the 2e-2 gate; fp8 e4m3 fails at ~1e-1 because the 40960-term
contraction amplifies quantization noise by sqrt(K)) and the features
are host-transposed/swizzled into the [k, b] layout the PE wants, so the
kernel is pure matmul: no on-chip transposes. Per core the two
[512, 40960] feature GEMMs read 105 MB of fp16 from HBM (~275-300 us at
the 340-400 GB/s per-NC rate) and cost 1280 N=512 matmuls x 216 ns on
the PE (~277 us at the 78.6 TF/s bf16-class rate) -- balanced at the
joint roofline. Features/weights stream in k-slabs of 4 k-tiles with
bufs=8 (DMA never waits on buffer-free), split across both HWDGE rings
(white on sync, black on scalar, W_ft alternating). W_ft.T tiles are
stationary, feat.T [k, b=512] moving, accumulating out.T [h, b] in PSUM
across all 320 k-tiles (4 banks: 2 sides x 2 h-tiles). Dummy warm-up
matmuls cover the ~8 us HWDGE bring-up so the HAM clock gate is at
2.4 GHz when real matmuls start. The clip is fused into the PSUM
evacuation (white's evacuation hides under black's final matmuls), the
tiny MLP stays in transposed [features, batch] layout, and b3 is folded
into the last matmul via a ones-row.

This walrus build rejects instructions with >1 sync wait, so a post-pass
(_split_multi_waits) redistributes Tile-emitted waits onto single-wait
no-ops.
"""

import sys
import types

import numpy as np


def _inject_ntff_hook():
    """Register the axon NTFF profile hook if this image's antenv lacks it."""
    try:
        import antenv.axon_hooks  # noqa: F401
        return
    except ImportError:
        pass
    try:
        import trn_agent_boot.trn_boot as tb
        hook = tb._ntff_profile_via_ctypes("/opt/axon/libaxon_pjrt.so")
    except Exception:
        hook = None
    mod = types.ModuleType("antenv.axon_hooks")
    mod.get_axon_ntff_profile_hook = lambda: hook
    mod.set_axon_ntff_profile_hook = lambda h: None
    sys.modules["antenv.axon_hooks"] = mod


_inject_ntff_hook()

import concourse.bass as bass
import concourse.mybir as mybir
from concourse.tile import TileContext

N_CORES = 8
B = 4096
BS = B // N_CORES          # 512 batch rows per core
IN = 40960                 # feature count (contraction dim)
H = 256                    # hidden per perspective
NKT = 4                    # k-tiles per slab
KC = NKT * 128             # k-slab width: 512
NSLAB = IN // KC           # 80
NKTOT = IN // 128          # 320 k-tiles total

F32 = mybir.dt.float32
F16 = mybir.dt.float16


def _split_multi_waits(nc: bass.Bass) -> None:
    """This walrus build rejects instructions carrying more than one sync
    wait. Split any such instruction: emit single-wait no-ops on the same
    engine immediately before it (same engine stream => same semantics)."""
    for f in nc.m.functions:
        for bb in f.blocks:
            new_insts = []
            changed = False
            for inst in bb.instructions:
                si = inst.sync_info
                waits = list(si.on_wait) if si is not None and si.on_wait else []
                if len(waits) > 1:
                    changed = True
                    for i, w in enumerate(waits[:-1]):
                        nop = mybir.InstNoOp(
                            name=f"{inst.name}-sw{i}", ins=[], outs=[]
                        )
                        nop.engine = inst.engine
                        nop.sync_info = mybir.SyncInfo(on_wait=[w], on_update=[])
                        nc.register_instruction(nop)
                        new_insts.append(nop)
                    inst.sync_info = mybir.SyncInfo(
                        on_wait=[waits[-1]],
                        on_update=list(si.on_update) if si.on_update else [],
                    )
                new_insts.append(inst)
            if changed:
                bb.instructions = new_insts


def build_kernel(mm_f32r: bool = True, tr_f32r: bool = True) -> bass.Bass:
    nc = bass.Bass()

    # Features arrive host-transposed+swizzled fp16: [128, NKTOT*BS] where
    # row p, columns [kt*BS : (kt+1)*BS] hold feat.T[kt*128 + p, :]. Each
    # k-slab DMA reads NKT*BS*2 = 4 KB contiguous per partition.
    wf = nc.dram_tensor("white_fT", [128, NKTOT * BS], F16, kind="ExternalInput")
    bf = nc.dram_tensor("black_fT", [128, NKTOT * BS], F16, kind="ExternalInput")
    # W_ft.T swizzled the same way: [128, NKTOT*H], 4 KB/partition per slab.
    w_ftTs = nc.dram_tensor("W_ftTs", [128, NKTOT * H], F16, kind="ExternalInput")
    w1Ts = nc.dram_tensor("W1Ts", [128, 128], F16, kind="ExternalInput")
    b1 = nc.dram_tensor("b1", [32, 1], F32, kind="ExternalInput")
    w2T = nc.dram_tensor("W2T", [32, 32], F16, kind="ExternalInput")
    b2 = nc.dram_tensor("b2", [32, 1], F32, kind="ExternalInput")
    # W3T has b3 folded in as a 33rd row (paired with a ones-row in h2)
    w3T = nc.dram_tensor("W3T", [33, 1], F16, kind="ExternalInput")
    stm = nc.dram_tensor("side_to_move", [1, BS], F32, kind="ExternalInput")
    out = nc.dram_tensor("evaluation", [1, BS], F32, kind="ExternalOutput")

    feats = [wf, bf]

    with TileContext(nc) as tc:
        with (
            tc.tile_pool(name="ot_psum", bufs=1, space="PSUM") as ot_pool,
            tc.tile_pool(name="mlp", bufs=1) as mlp,
        ):
            # out.T accumulators: [h-tile 128, b 512] x (2 sides x 2 h-tiles)
            ot = [
                ot_pool.tile([128, BS], F32, tag=f"ot{i}", name=f"ot{i}")
                for i in range(4)
            ]
            xt = []  # clipped fp16 copies, filled during the last slab

            # ---- PE warm-up: the HAM clock gate defaults to 1.2 GHz and
            # only lifts to 2.4 GHz after ~3.4us of sustained PE activity.
            # Burn that window on dummy matmuls while the first feature
            # slabs are still in flight, so real matmuls start warm.
            with (
                tc.tile_pool(name="warm", bufs=1) as warm_pool,
                tc.tile_pool(name="warm_psum", bufs=1, space="PSUM") as wp_pool,
            ):
                dum_w = warm_pool.tile([128, 128], F16)
                nc.vector.memset(dum_w[:], 0.0)
                dum_f = warm_pool.tile([128, BS], F16)
                nc.vector.memset(dum_f[:], 0.0)
                # ~12 cold (427ns) + 2 warm (216ns) spans ~5.6us; the first
                # feature slab lands ~7.5us, leaving a <2us idle gap --
                # short enough that the clock gate stays at 8/8.
                dum_o = wp_pool.tile([128, BS], F32)
                for _ in range(14):
                    nc.tensor.matmul(
                        dum_o, dum_w[:], dum_f[:], start=True, stop=True
                    )

            # ---- main loop: feature-transformer GEMMs ----
            # slab widths in k-tiles: two small warmup slabs so the PE
            # starts early, then uniform NKT-wide slabs.
            widths = [1, 1, 2] + [NKT] * ((NKTOT - 4) // NKT)
            assert sum(widths) == NKTOT
            with (
                tc.tile_pool(name="fslab", bufs=8) as fslab_pool,
                tc.tile_pool(name="wt", bufs=8) as wt_pool,
                tc.tile_pool(name="pre", bufs=1) as pre_pool,
            ):
                # white features ride the sync HWDGE ring, black the
                # scalar ring; W_ft alternates so both rings carry ~52 MB.
                f_eng = [nc.sync, nc.scalar]
                kt0 = 0
                # W is fetched in 2-slab (8-tile, 4 KB/partition) chunks so
                # its DMA packets hit the 4 KB descriptor-overhead floor;
                # features stay at 4-tile slabs for pipeline granularity.
                wt_cur = None
                pair_idx = 0
                for s, w in enumerate(widths):
                    pre = w != NKT
                    last_slab = s == len(widths) - 1
                    pool = pre_pool if pre else fslab_pool
                    if pre or (wt_cur is None and last_slab):
                        wt_use = pre_pool.tile(
                            [128, w, H], F16, tag=f"wt{s}", name="wt")
                        off = 0
                        f_eng[s % 2].dma_start(
                            out=wt_use[:],
                            in_=w_ftTs[:, kt0 * H:(kt0 + w) * H],
                        )
                    elif wt_cur is None:
                        wt_use = wt_pool.tile(
                            [128, 2 * NKT, H], F16, tag="wt", name="wt")
                        off = 0
                        f_eng[pair_idx % 2].dma_start(
                            out=wt_use[:],
                            in_=w_ftTs[:, kt0 * H:(kt0 + 2 * NKT) * H],
                        )
                        wt_cur = wt_use
                        pair_idx += 1
                    else:
                        wt_use, off = wt_cur, NKT
                        wt_cur = None
                    wt = wt_use
                    fsl = []
                    for side in range(2):
                        f_t = pool.tile(
                            [128, w, BS], F16,
                            tag=f"pre{side}_{s}" if pre else f"fslab{side}",
                            name=f"fsl{side}",
                        )
                        f_eng[side].dma_start(
                            out=f_t[:],
                            in_=feats[side][
                                :, kt0 * BS:(kt0 + w) * BS
                            ],
                        )
                        fsl.append(f_t)

                    last_slab = s == len(widths) - 1
                    if not last_slab:
                        for kt in range(w):
                            first = kt0 == 0 and kt == 0
                            for h in range(2):
                                for side in range(2):
                                    nc.tensor.matmul(
                                        ot[side * 2 + h],
                                        wt[:, off + kt, h * 128:(h + 1) * 128],
                                        fsl[side][:, kt, :],
                                        start=first,
                                        stop=False,
                                    )
                    else:
                        # final slab: finish white first, evacuate its
                        # PSUM banks while black's last matmuls run.
                        for side in range(2):
                            for kt in range(w):
                                for h in range(2):
                                    nc.tensor.matmul(
                                        ot[side * 2 + h],
                                        wt[:, off + kt, h * 128:(h + 1) * 128],
                                        fsl[side][:, kt, :],
                                        start=False,
                                        stop=kt == w - 1,
                                    )
                            for i in range(2 * side, 2 * side + 2):
                                t = mlp.tile([128, BS], F16, tag=f"xt{i}",
                                             name="xt")
                                # white: one full evac (hides under
                                # black's matmuls). black: halves, so
                                # the MLP can start on half 0 while
                                # half 1 still evacuates.
                                nsp = 1 if side == 0 else 2
                                for sp in range(nsp):
                                    sl = slice(sp * (BS // nsp),
                                               (sp + 1) * (BS // nsp))
                                    nc.vector.tensor_scalar(
                                        out=t[:, sl], in0=ot[i][:, sl],
                                        scalar1=0.0, scalar2=1.0,
                                        op0=mybir.AluOpType.max,
                                        op1=mybir.AluOpType.min,
                                    )
                                xt.append(t)
                    kt0 += w

            # ---- MLP weight prep (emitted late so these DMAs schedule
            # behind the feature stream, not ahead of it) ----
            w1t = mlp.tile([128, 4, 32], F16)
            nc.scalar.dma_start(out=w1t[:], in_=w1Ts[:, :])
            w2t = mlp.tile([32, 32], F16)
            nc.scalar.dma_start(out=w2t[:], in_=w2T[:, :])
            w3t = mlp.tile([33, 1], F16)
            nc.scalar.dma_start(out=w3t[:], in_=w3T[:, :])
            b1_sb = mlp.tile([32, 1], F32)
            nc.scalar.dma_start(out=b1_sb[:], in_=b1[:, :])
            b2_sb = mlp.tile([32, 1], F32)
            nc.scalar.dma_start(out=b2_sb[:], in_=b2[:, :])
            stm_sb = mlp.tile([1, BS], F32)
            nc.scalar.dma_start(out=stm_sb[:], in_=stm[:, :])
            # h2 carries a ones-row (partition 32) so the final matmul
            # against [W3.T; b3] folds the bias in.
            h2 = mlp.tile([33, BS], F16)
            nc.vector.memset(h2[32:33, :], 1.0)

            # ---- MLP (transposed layout throughout; xt built above).
            # The whole chain runs in two 256-column halves with
            # independent PSUM groups, so DVE evacuations of one half
            # pipeline with PE matmuls of the other. ----
            with tc.tile_pool(name="mlp2_psum", bufs=1, space="PSUM") as mpp2:
                h1p = mpp2.tile([32, BS], F32, tag="h1")
                h1 = mlp.tile([32, BS], F16)
                h2p = mpp2.tile([32, BS], F32, tag="h2")
                evp = mpp2.tile([1, BS], F32, tag="ev")
                HB = BS // 2
                for hf in range(2):
                    sl = slice(hf * HB, (hf + 1) * HB)
                    for kt in range(4):
                        nc.tensor.matmul(
                            h1p[:, sl], w1t[:, kt, :], xt[kt][:, sl],
                            start=kt == 0, stop=kt == 3,
                        )
                    nc.vector.tensor_scalar(
                        out=h1[:, sl], in0=h1p[:, sl], scalar1=b1_sb[:, :],
                        scalar2=0.0,
                        op0=mybir.AluOpType.add, op1=mybir.AluOpType.max,
                    )
                    nc.tensor.matmul(
                        h2p[:, sl], w2t[:], h1[:, sl], start=True, stop=True
                    )
                    nc.vector.tensor_scalar(
                        out=h2[0:32, sl], in0=h2p[:, sl], scalar1=b2_sb[:, :],
                        scalar2=0.0,
                        op0=mybir.AluOpType.add, op1=mybir.AluOpType.max,
                    )
                    nc.tensor.matmul(
                        evp[:, sl], w3t[:], h2[:, sl], start=True, stop=True
                    )
                evs = mlp.tile([1, BS], F32)
                nc.vector.tensor_mul(out=evs[:], in0=evp[:], in1=stm_sb[:])
                nc.sync.dma_start(out=out[:, :], in_=evs[:])

    _split_multi_waits(nc)
    return nc


_NC_CACHE: dict = {}


def _get_nc(mm_f32r: bool = True, tr_f32r: bool = True) -> bass.Bass:
    key = (mm_f32r, tr_f32r)
    if key not in _NC_CACHE:
        _NC_CACHE[key] = build_kernel(mm_f32r=mm_f32r, tr_f32r=tr_f32r)
    return _NC_CACHE[key]


def _swizzle_T(arr_f16: np.ndarray, ncols: int) -> np.ndarray:
    """[rows, IN] fp16 -> [128, NKTOT*rows] where row p, cols
    [kt*rows:(kt+1)*rows] = arr.T[kt*128 + p, :]."""
    rows = arr_f16.shape[0]
    assert arr_f16.shape == (rows, IN) and ncols == rows
    return np.ascontiguousarray(
        arr_f16.reshape(rows, NKTOT, 128).transpose(2, 1, 0)
    ).reshape(128, NKTOT * rows)


def make_in_maps(inputs: dict) -> list:
    """Shard full inputs into per-core input maps (fp16, transposed)."""
    wf = np.asarray(inputs["white_features"]).astype(np.float16)
    bf = np.asarray(inputs["black_features"]).astype(np.float16)
    stm = np.ascontiguousarray(inputs["side_to_move"], dtype=np.float32)
    w_ftTs = _swizzle_T(
        np.asarray(inputs["W_ft"], dtype=np.float32).astype(np.float16), H)
    w1T = np.asarray(inputs["W1"], dtype=np.float32).astype(np.float16).T
    w1Ts = np.ascontiguousarray(
        w1T.reshape(4, 128, 32).transpose(1, 0, 2)).reshape(128, 128)
    w2T = np.ascontiguousarray(
        np.asarray(inputs["W2"], dtype=np.float32).astype(np.float16).T)
    w3T = np.concatenate([
        np.asarray(inputs["W3"], dtype=np.float32).astype(np.float16).T,
        np.asarray(inputs["b3"], dtype=np.float32).astype(np.float16)
        .reshape(1, 1),
    ], axis=0)  # [33, 1]: W3.T with b3 folded in
    maps = []
    for c in range(N_CORES):
        sl = slice(c * BS, (c + 1) * BS)
        maps.append({
            "white_fT": _swizzle_T(wf[sl], BS),
            "black_fT": _swizzle_T(bf[sl], BS),
            "side_to_move": stm[sl].reshape(1, BS),
            "W_ftTs": w_ftTs,
            "W1Ts": w1Ts,
            "b1": np.ascontiguousarray(inputs["b1"], dtype=np.float32).reshape(32, 1),
            "W2T": w2T,
            "b2": np.ascontiguousarray(inputs["b2"], dtype=np.float32).reshape(32, 1),
            "W3T": w3T,
        })
    return maps


def run(inputs: dict, trace: bool = False, mm_f32r: bool = True,
        tr_f32r: bool = True):
    """Run on all 8 cores; returns (full_output [4096,1] fp32, BassKernelResults)."""
    from concourse.bass_utils import run_bass_kernel_spmd

    nc = _get_nc(mm_f32r=mm_f32r, tr_f32r=tr_f32r)
    res = run_bass_kernel_spmd(
        nc, make_in_maps(inputs), core_ids=list(range(N_CORES)), trace=trace
    )
    full = np.concatenate(
        [res.results[c]["evaluation"].reshape(BS, 1) for c in range(N_CORES)],
        axis=0,
    ).astype(np.float32)
    return full, res


def kernel(**inputs) -> np.ndarray:
    return run(inputs, trace=False)[0]


if __name__ == "__main__":
    rng = np.random.default_rng(0)
    ins = {
        "white_features": rng.random((B, IN), dtype=np.float32),
        "black_features": rng.random((B, IN), dtype=np.float32),
        "side_to_move": np.ones((B,), dtype=np.float32),
        "W_ft": (0.1 * rng.standard_normal((H, IN))).astype(np.float32),
        "W1": (0.06 * rng.standard_normal((32, 2 * H))).astype(np.float32),
        "b1": np.zeros(32, np.float32),
        "W2": (0.17 * rng.standard_normal((32, 32))).astype(np.float32),
        "b2": np.zeros(32, np.float32),
        "W3": (0.24 * rng.standard_normal((1, 32))).astype(np.float32),
        "b3": np.zeros(1, np.float32),
    }
    out = kernel(**ins)
    # host reference
    whr = np.clip(ins["white_features"] @ ins["W_ft"].T, 0, 1)
    bhr = np.clip(ins["black_features"] @ ins["W_ft"].T, 0, 1)
    x = np.concatenate([whr, bhr], axis=1)
    x = np.maximum(x @ ins["W1"].T + ins["b1"], 0)
    x = np.maximum(x @ ins["W2"].T + ins["b2"], 0)
    ref = (x @ ins["W3"].T + ins["b3"]) * ins["side_to_move"][:, None]
    rel = np.linalg.norm(out - ref) / np.linalg.norm(ref)
    print("rel err:", rel)


# revision 37
# speedup vs baseline: 1.0684x; 1.0009x over previous
"""NNUE evaluation kernel for Trainium2 (8 NeuronCores, data-parallel batch).

reference math:
    wh = clip(white @ W_ft.T, 0, 1)        # [B, 256]
    bh = clip(black @ W_ft.T, 0, 1)        # [B, 256]
    x  = concat(wh, bh)                    # [B, 512]
    x  = relu(x @ W1.T + b1); x = relu(x @ W2.T + b2)
    ev = (x @ W3.T + b3) * stm[:, None]    # [B, 1]

Strategy: shard B=4096 across 8 cores (512 rows each), data-parallel, no
collectives. All GEMM operands are cast to fp16 on the host (rel err
1.7e-3 vs the 2e-2 gate; fp8 e4m3 fails at ~1e-1 because the 40960-term
contraction amplifies quantization noise by sqrt(K)) and the features
are host-transposed/swizzled into the [k, b] layout the PE wants, so the
kernel is pure matmul: no on-chip transposes. Per core the two
[512, 40960] feature GEMMs read 105 MB of fp16 from HBM (~275-300 us at
the 340-400 GB/s per-NC rate) and cost 1280 N=512 matmuls x 216 ns on
the PE (~277 us at the 78.6 TF/s bf16-class rate) -- balanced at the
joint roofline. Features/weights stream in k-slabs of 4 k-tiles with
bufs=8 (DMA never waits on buffer-free), split across both HWDGE rings
(white on sync, black on scalar, W_ft alternating). W_ft.T tiles are
stationary, feat.T [k, b=512] moving, accumulating out.T [h, b] in PSUM
across all 320 k-tiles (4 banks: 2 sides x 2 h-tiles). Dummy warm-up
matmuls cover the ~8 us HWDGE bring-up so the HAM clock gate is at
2.4 GHz when real matmuls start. The clip is fused into the PSUM
evacuation (white's evacuation hides under black's final matmuls), the
tiny MLP stays in transposed [features, batch] layout, and b3 is folded
into the last matmul via a ones-row.

This walrus build rejects instructions with >1 sync wait, so a post-pass
(_split_multi_waits) redistributes Tile-emitted waits onto single-wait
no-ops.
"""

import sys
import types

import numpy as np


def _inject_ntff_hook():
    """Register the axon NTFF profile hook if this image's antenv lacks it."""
    try:
        import antenv.axon_hooks  # noqa: F401
        return
    except ImportError:
        pass
    try:
        import trn_agent_boot.trn_boot as tb
        hook = tb._ntff_profile_via_ctypes("/opt/axon/libaxon_pjrt.so")
    except Exception:
        hook = None
    mod = types.ModuleType("antenv.axon_hooks")
    mod.get_axon_ntff_profile_hook = lambda: hook
    mod.set_axon_ntff_profile_hook = lambda h: None
    sys.modules["antenv.axon_hooks"] = mod


_inject_ntff_hook()

import concourse.bass as bass
import concourse.mybir as mybir
from concourse.tile import TileContext

N_CORES = 8
B = 4096
BS = B // N_CORES          # 512 batch rows per core
IN = 40960                 # feature count (contraction dim)
H = 256                    # hidden per perspective
NKT = 4                    # k-tiles per slab
KC = NKT * 128             # k-slab width: 512
NSLAB = IN // KC           # 80
NKTOT = IN // 128          # 320 k-tiles total

F32 = mybir.dt.float32
F16 = mybir.dt.float16


def _split_multi_waits(nc: bass.Bass) -> None:
    """This walrus build rejects instructions carrying more than one sync
    wait. Split any such instruction: emit single-wait no-ops on the same
    engine immediately before it (same engine stream => same semantics)."""
    for f in nc.m.functions:
        for bb in f.blocks:
            new_insts = []
            changed = False
            for inst in bb.instructions:
                si = inst.sync_info
                waits = list(si.on_wait) if si is not None and si.on_wait else []
                if len(waits) > 1:
                    changed = True
                    for i, w in enumerate(waits[:-1]):
                        nop = mybir.InstNoOp(
                            name=f"{inst.name}-sw{i}", ins=[], outs=[]
                        )
                        nop.engine = inst.engine
                        nop.sync_info = mybir.SyncInfo(on_wait=[w], on_update=[])
                        nc.register_instruction(nop)
                        new_insts.append(nop)
                    inst.sync_info = mybir.SyncInfo(
                        on_wait=[waits[-1]],
                        on_update=list(si.on_update) if si.on_update else [],
                    )
                new_insts.append(inst)
            if changed:
                bb.instructions = new_insts


def build_kernel(mm_f32r: bool = True, tr_f32r: bool = True) -> bass.Bass:
    nc = bass.Bass()

    # Features arrive host-transposed+swizzled fp16: [128, NKTOT*BS] where
    # row p, columns [kt*BS : (kt+1)*BS] hold feat.T[kt*128 + p, :]. Each
    # k-slab DMA reads NKT*BS*2 = 4 KB contiguous per partition.
    wf = nc.dram_tensor("white_fT", [128, NKTOT * BS], F16, kind="ExternalInput")
    bf = nc.dram_tensor("black_fT", [128, NKTOT * BS], F16, kind="ExternalInput")
    # W_ft.T swizzled the same way: [128, NKTOT*H], 4 KB/partition per slab.
    w_ftTs = nc.dram_tensor("W_ftTs", [128, NKTOT * H], F16, kind="ExternalInput")
    w1Ts = nc.dram_tensor("W1Ts", [128, 128], F16, kind="ExternalInput")
    b1 = nc.dram_tensor("b1", [32, 1], F32, kind="ExternalInput")
    w2T = nc.dram_tensor("W2T", [32, 32], F16, kind="ExternalInput")
    b2 = nc.dram_tensor("b2", [32, 1], F32, kind="ExternalInput")
    # W3T has b3 folded in as a 33rd row (paired with a ones-row in h2)
    w3T = nc.dram_tensor("W3T", [33, 1], F16, kind="ExternalInput")
    stm = nc.dram_tensor("side_to_move", [1, BS], F32, kind="ExternalInput")
    out = nc.dram_tensor("evaluation", [1, BS], F32, kind="ExternalOutput")

    feats = [wf, bf]

    with TileContext(nc) as tc:
        with (
            tc.tile_pool(name="ot_psum", bufs=1, space="PSUM") as ot_pool,
            tc.tile_pool(name="mlp", bufs=1) as mlp,
        ):
            # out.T accumulators: [h-tile 128, b 512] x (2 sides x 2 h-tiles)
            ot = [
                ot_pool.tile([128, BS], F32, tag=f"ot{i}", name=f"ot{i}")
                for i in range(4)
            ]
            xt = []  # clipped fp16 copies, filled during the last slab

            # ---- PE warm-up: the HAM clock gate defaults to 1.2 GHz and
            # only lifts to 2.4 GHz after ~3.4us of sustained PE activity.
            # Burn that window on dummy matmuls while the first feature
            # slabs are still in flight, so real matmuls start warm.
            with (
                tc.tile_pool(name="warm", bufs=1) as warm_pool,
                tc.tile_pool(name="warm_psum", bufs=1, space="PSUM") as wp_pool,
            ):
                dum_w = warm_pool.tile([128, 128], F16)
                nc.vector.memset(dum_w[:], 0.0)
                dum_f = warm_pool.tile([128, BS], F16)
                nc.vector.memset(dum_f[:], 0.0)
                # ~12 cold (427ns) + 2 warm (216ns) spans ~5.6us; the first
                # feature slab lands ~7.5us, leaving a <2us idle gap --
                # short enough that the clock gate stays at 8/8.
                dum_o = wp_pool.tile([128, BS], F32)
                for _ in range(14):
                    nc.tensor.matmul(
                        dum_o, dum_w[:], dum_f[:], start=True, stop=True
                    )

            # ---- main loop: feature-transformer GEMMs ----
            # slab widths in k-tiles: two small warmup slabs so the PE
            # starts early, then uniform NKT-wide slabs.
            widths = [1, 1, 2] + [NKT] * ((NKTOT - 4) // NKT)
            assert sum(widths) == NKTOT
            with (
                tc.tile_pool(name="fslab", bufs=12) as fslab_pool,
                tc.tile_pool(name="wt", bufs=8) as wt_pool,
                tc.tile_pool(name="pre", bufs=1) as pre_pool,
            ):
                # white features ride the sync HWDGE ring, black the
                # scalar ring; W_ft alternates so both rings carry ~52 MB.
                f_eng = [nc.sync, nc.scalar]
                kt0 = 0
                # W is fetched in 2-slab (8-tile, 4 KB/partition) chunks so
                # its DMA packets hit the 4 KB descriptor-overhead floor;
                # features stay at 4-tile slabs for pipeline granularity.
                wt_cur = None
                pair_idx = 0
                for s, w in enumerate(widths):
                    pre = w != NKT
                    last_slab = s == len(widths) - 1
                    pool = pre_pool if pre else fslab_pool
                    if pre or (wt_cur is None and last_slab):
                        wt_use = pre_pool.tile(
                            [128, w, H], F16, tag=f"wt{s}", name="wt")
                        off = 0
                        f_eng[s % 2].dma_start(
                            out=wt_use[:],
                            in_=w_ftTs[:, kt0 * H:(kt0 + w) * H],
                        )
                    elif wt_cur is None:
                        wt_use = wt_pool.tile(
                            [128, 2 * NKT, H], F16, tag="wt", name="wt")
                        off = 0
                        f_eng[pair_idx % 2].dma_start(
                            out=wt_use[:],
                            in_=w_ftTs[:, kt0 * H:(kt0 + 2 * NKT) * H],
                        )
                        wt_cur = wt_use
                        pair_idx += 1
                    else:
                        wt_use, off = wt_cur, NKT
                        wt_cur = None
                    wt = wt_use
                    fsl = []
                    for side in range(2):
                        f_t = pool.tile(
                            [128, w, BS], F16,
                            tag=f"pre{side}_{s}" if pre else f"fslab{side}",
                            name=f"fsl{side}",
                        )
                        f_eng[side].dma_start(
                            out=f_t[:],
                            in_=feats[side][
                                :, kt0 * BS:(kt0 + w) * BS
                            ],
                        )
                        fsl.append(f_t)

                    last_slab = s == len(widths) - 1
                    if not last_slab:
                        for kt in range(w):
                            first = kt0 == 0 and kt == 0
                            for h in range(2):
                                for side in range(2):
                                    nc.tensor.matmul(
                                        ot[side * 2 + h],
                                        wt[:, off + kt, h * 128:(h + 1) * 128],
                                        fsl[side][:, kt, :],
                                        start=first,
                                        stop=False,
                                    )
                    else:
                        # final slab: finish white first, evacuate its
                        # PSUM banks while black's last matmuls run.
                        for side in range(2):
                            for kt in range(w):
                                for h in range(2):
                                    nc.tensor.matmul(
                                        ot[side * 2 + h],
                                        wt[:, off + kt, h * 128:(h + 1) * 128],
                                        fsl[side][:, kt, :],
                                        start=False,
                                        stop=kt == w - 1,
                                    )
                            for i in range(2 * side, 2 * side + 2):
                                t = mlp.tile([128, BS], F16, tag=f"xt{i}",
                                             name="xt")
                                # white: one full evac (hides under
                                # black's matmuls). black: halves, so
                                # the MLP can start on half 0 while
                                # half 1 still evacuates.
                                nsp = 1 if side == 0 else 2
                                for sp in range(nsp):
                                    sl = slice(sp * (BS // nsp),
                                               (sp + 1) * (BS // nsp))
                                    nc.vector.tensor_scalar(
                                        out=t[:, sl], in0=ot[i][:, sl],
                                        scalar1=0.0, scalar2=1.0,
                                        op0=mybir.AluOpType.max,
                                        op1=mybir.AluOpType.min,
                                    )
                                xt.append(t)
                    kt0 += w

            # ---- MLP weight prep (emitted late so these DMAs schedule
            # behind the feature stream, not ahead of it) ----
            w1t = mlp.tile([128, 4, 32], F16)
            nc.scalar.dma_start(out=w1t[:], in_=w1Ts[:, :])
            w2t = mlp.tile([32, 32], F16)
            nc.scalar.dma_start(out=w2t[:], in_=w2T[:, :])
            w3t = mlp.tile([33, 1], F16)
            nc.scalar.dma_start(out=w3t[:], in_=w3T[:, :])
            b1_sb = mlp.tile([32, 1], F32)
            nc.scalar.dma_start(out=b1_sb[:], in_=b1[:, :])
            b2_sb = mlp.tile([32, 1], F32)
            nc.scalar.dma_start(out=b2_sb[:], in_=b2[:, :])
            stm_sb = mlp.tile([1, BS], F32)
            nc.scalar.dma_start(out=stm_sb[:], in_=stm[:, :])
            # h2 carries a ones-row (partition 32) so the final matmul
            # against [W3.T; b3] folds the bias in.
            h2 = mlp.tile([33, BS], F16)
            nc.vector.memset(h2[32:33, :], 1.0)

            # ---- MLP (transposed layout throughout; xt built above).
            # The whole chain runs in two 256-column halves with
            # independent PSUM groups, so DVE evacuations of one half
            # pipeline with PE matmuls of the other. ----
            with tc.tile_pool(name="mlp2_psum", bufs=1, space="PSUM") as mpp2:
                h1p = mpp2.tile([32, BS], F32, tag="h1")
                h1 = mlp.tile([32, BS], F16)
                h2p = mpp2.tile([32, BS], F32, tag="h2")
                evp = mpp2.tile([1, BS], F32, tag="ev")
                HB = BS // 2
                for hf in range(2):
                    sl = slice(hf * HB, (hf + 1) * HB)
                    for kt in range(4):
                        nc.tensor.matmul(
                            h1p[:, sl], w1t[:, kt, :], xt[kt][:, sl],
                            start=kt == 0, stop=kt == 3,
                        )
                    nc.vector.tensor_scalar(
                        out=h1[:, sl], in0=h1p[:, sl], scalar1=b1_sb[:, :],
                        scalar2=0.0,
                        op0=mybir.AluOpType.add, op1=mybir.AluOpType.max,
                    )
                    nc.tensor.matmul(
                        h2p[:, sl], w2t[:], h1[:, sl], start=True, stop=True
                    )
                    nc.vector.tensor_scalar(
                        out=h2[0:32, sl], in0=h2p[:, sl], scalar1=b2_sb[:, :],
                        scalar2=0.0,
                        op0=mybir.AluOpType.add, op1=mybir.AluOpType.max,
                    )
                    nc.tensor.matmul(
                        evp[:, sl], w3t[:], h2[:, sl], start=True, stop=True
                    )
                evs = mlp.tile([1, BS], F32)
                nc.vector.tensor_mul(out=evs[:], in0=evp[:], in1=stm_sb[:])
                nc.sync.dma_start(out=out[:, :], in_=evs[:])

    _split_multi_waits(nc)
    return nc


_NC_CACHE: dict = {}


def _get_nc(mm_f32r: bool = True, tr_f32r: bool = True) -> bass.Bass:
    key = (mm_f32r, tr_f32r)
    if key not in _NC_CACHE:
        _NC_CACHE[key] = build_kernel(mm_f32r=mm_f32r, tr_f32r=tr_f32r)
    return _NC_CACHE[key]


def _swizzle_T(arr_f16: np.ndarray, ncols: int) -> np.ndarray:
    """[rows, IN] fp16 -> [128, NKTOT*rows] where row p, cols
    [kt*rows:(kt+1)*rows] = arr.T[kt*128 + p, :]."""
    rows = arr_f16.shape[0]
    assert arr_f16.shape == (rows, IN) and ncols == rows
    return np.ascontiguousarray(
        arr_f16.reshape(rows, NKTOT, 128).transpose(2, 1, 0)
    ).reshape(128, NKTOT * rows)


def make_in_maps(inputs: dict) -> list:
    """Shard full inputs into per-core input maps (fp16, transposed)."""
    wf = np.asarray(inputs["white_features"]).astype(np.float16)
    bf = np.asarray(inputs["black_features"]).astype(np.float16)
    stm = np.ascontiguousarray(inputs["side_to_move"], dtype=np.float32)
    w_ftTs = _swizzle_T(
        np.asarray(inputs["W_ft"], dtype=np.float32).astype(np.float16), H)
    w1T = np.asarray(inputs["W1"], dtype=np.float32).astype(np.float16).T
    w1Ts = np.ascontiguousarray(
        w1T.reshape(4, 128, 32).transpose(1, 0, 2)).reshape(128, 128)
    w2T = np.ascontiguousarray(
        np.asarray(inputs["W2"], dtype=np.float32).astype(np.float16).T)
    w3T = np.concatenate([
        np.asarray(inputs["W3"], dtype=np.float32).astype(np.float16).T,
        np.asarray(inputs["b3"], dtype=np.float32).astype(np.float16)
        .reshape(1, 1),
    ], axis=0)  # [33, 1]: W3.T with b3 folded in
    maps = []
    for c in range(N_CORES):
        sl = slice(c * BS, (c + 1) * BS)
        maps.append({
            "white_fT": _swizzle_T(wf[sl], BS),
            "black_fT": _swizzle_T(bf[sl], BS),
            "side_to_move": stm[sl].reshape(1, BS),
            "W_ftTs": w_ftTs,
            "W1Ts": w1Ts,
            "b1": np.ascontiguousarray(inputs["b1"], dtype=np.float32).reshape(32, 1),
            "W2T": w2T,
            "b2": np.ascontiguousarray(inputs["b2"], dtype=np.float32).reshape(32, 1),
            "W3T": w3T,
        })
    return maps


def run(inputs: dict, trace: bool = False, mm_f32r: bool = True,
        tr_f32r: bool = True):
    """Run on all 8 cores; returns (full_output [4096,1] fp32, BassKernelResults)."""
    from concourse.bass_utils import run_bass_kernel_spmd

    nc = _get_nc(mm_f32r=mm_f32r, tr_f32r=tr_f32r)
    res = run_bass_kernel_spmd(
        nc, make_in_maps(inputs), core_ids=list(range(N_CORES)), trace=trace
    )
    full = np.concatenate(
        [res.results[c]["evaluation"].reshape(BS, 1) for c in range(N_CORES)],
        axis=0,
    ).astype(np.float32)
    return full, res


def kernel(**inputs) -> np.ndarray:
    return run(inputs, trace=False)[0]


if __name__ == "__main__":
    rng = np.random.default_rng(0)
    ins = {
        "white_features": rng.random((B, IN), dtype=np.float32),
        "black_features": rng.random((B, IN), dtype=np.float32),
        "side_to_move": np.ones((B,), dtype=np.float32),
        "W_ft": (0.1 * rng.standard_normal((H, IN))).astype(np.float32),
        "W1": (0.06 * rng.standard_normal((32, 2 * H))).astype(np.float32),
        "b1": np.zeros(32, np.float32),
        "W2": (0.17 * rng.standard_normal((32, 32))).astype(np.float32),
        "b2": np.zeros(32, np.float32),
        "W3": (0.24 * rng.standard_normal((1, 32))).astype(np.float32),
        "b3": np.zeros(1, np.float32),
    }
    out = kernel(**ins)
    # host reference
    whr = np.clip(ins["white_features"] @ ins["W_ft"].T, 0, 1)
    bhr = np.clip(ins["black_features"] @ ins["W_ft"].T, 0, 1)
    x = np.concatenate([whr, bhr], axis=1)
    x = np.maximum(x @ ins["W1"].T + ins["b1"], 0)
    x = np.maximum(x @ ins["W2"].T + ins["b2"], 0)
    ref = (x @ ins["W3"].T + ins["b3"]) * ins["side_to_move"][:, None]
    rel = np.linalg.norm(out - ref) / np.linalg.norm(ref)
    print("rel err:", rel)
